# revision 1
# baseline (speedup 1.0000x reference)
"""Trainium2 Bass kernel for the NMS-BP decoder — PE-routed (bf16-triple) edition.

Self-contained: takes the FULL inputs of reference.setup_inputs(), shards the
batch across 8 NeuronCores (pure data parallelism), runs a Bass/Tile NEFF per
core, and reassembles the full [6, 64, 1024] output.

Per core (B_local = 8) the whole decoder lives in SBUF/PSUM. The two sparse
routings per iteration (dense temp -> slots, slots -> column sums) run on the
TENSOR engine as one-hot matmuls:

  * weights = 0/1 incidence tiles in fp8e4 (exact);
  * moving data = bf16 TRIPLE (h, m, l) packed in the free dim: x = h+m+l
    reconstructs fp32 bit-exactly (3x8 significand bits), and bf16 keeps the
    full fp32 exponent range so no component goes subnormal (fp16 pairs hit
    the PE denormal path at ~100x cost);
  * PSUM accumulates in fp32; every product is 1.0 * bf16 so routing is exact.

Checks are reassigned to (mhi, mlow) positions sorted by mean column index,
which concentrates each q-plane's columns into few 128-column chunks: only
~104 of 192 (q, k) incidence tiles are nonzero and empty tiles are skipped.

The 12-comparator 6-lane sorting network runs as 5 fused layers (13 wide DVE
ops instead of 24 narrow ones); physical j-planes hold logical edge lanes in
LOGMAP order so layer 1 is a contiguous half-vs-half min/max. abs/sign run as
single Activation-engine ops, the sign products and sign*psign on GpSimd, and
the w_k scalings as scaled Activation copies, so DVE keeps only the critical
chain.

Layouts:
  check/slot domain: [128 p = mlow, 24 q = jp*4 + mhi, 8 b]; slot s = q*128+p,
  col(s) = row_cols[assign[(q%4)*128 + p], LOGMAP[q//4]].
  column domain:     [128 p = nlow, 8 k, 8 b]; column n = k*128 + p.
"""

import numpy as np

B, N, M, DC, NUM_ITERS = 64, 1024, 512, 6, 5
NCORES = 8
BL = B // NCORES          # 8 batch rows per core
NSLOT = M * DC            # 3072
LOGMAP = [0, 1, 2, 5, 3, 4]   # physical j-plane -> logical (sorted-col) lane

_CACHE = {}


def _layout(row_cols):
    """Check assignment (sorted by mean col) + per-slot columns + tile lists."""
    assign = np.argsort(row_cols.mean(axis=1), kind="stable")  # position -> check
    cols = np.empty(NSLOT, np.int64)
    for q in range(24):
        jp, mhi = q // 4, q % 4
        j = LOGMAP[jp]
        for p in range(128):
            cols[q * 128 + p] = row_cols[assign[mhi * 128 + p], j]
    present = [sorted({int(c) // 128 for c in cols[q * 128:(q + 1) * 128]})
               for q in range(24)]
    gt = [(q, k) for q in range(24) for k in present[q]]           # gather tiles
    ct = [(k, q) for k in range(8) for q in range(24) if k in present[q]]
    return assign, cols, present, gt, ct


def _weights(cols, gt, ct):
    wg = np.zeros((128, len(gt), 128), np.float32)
    for t, (q, k) in enumerate(gt):
        for po in range(128):
            c = cols[q * 128 + po]
            if c // 128 == k:
                wg[c % 128, t, po] = 1.0
    wc = np.zeros((128, len(ct), 128), np.float32)
    for t, (k, q) in enumerate(ct):
        for ps in range(128):
            c = cols[q * 128 + ps]
            if c // 128 == k:
                wc[ps, t, c % 128] = 1.0
    return wg, wc


def _build(cols, w, sp1, sp2, gt, ct):
    import concourse.bass as bass
    import concourse.bacc as bacc
    import concourse.tile as tile
    import concourse.mybir as mybir

    dt = mybir.dt
    Alu = mybir.AluOpType
    ActF = mybir.ActivationFunctionType
    f32 = dt.float32
    bf16 = dt.bfloat16
    f8 = dt.float8e4

    nc = bacc.Bacc("TRN2", target_bir_lowering=False, debug=False)

    NGT, NCT = len(gt), len(ct)
    soft_t = nc.dram_tensor("soft_t", [N, BL], f32, kind="ExternalInput")
    wg_d = nc.dram_tensor("wg", [128, NGT * 128], f8, kind="ExternalInput")
    wc_d = nc.dram_tensor("wc", [128, NCT * 128], f8, kind="ExternalInput")
    out = nc.dram_tensor("out", [NUM_ITERS + 1, N, BL], f32, kind="ExternalOutput")

    w = [float(x) for x in w]
    sp1 = float(sp1)
    sp2 = float(sp2)

    gt_pos = {qk: t for t, qk in enumerate(gt)}
    ct_pos = {kq: t for t, kq in enumerate(ct)}
    pres_q = {}
    for (q, k) in gt:
        pres_q.setdefault(q, []).append(k)
    pres_k = {}
    for (k, q) in ct:
        pres_k.setdefault(k, []).append(q)

    with tile.TileContext(nc) as tc:
        with (
            tc.tile_pool(name="const", bufs=1) as pc,
            tc.tile_pool(name="work", bufs=2) as pw,
            tc.tile_pool(name="srt", bufs=12) as psrt,
            tc.tile_pool(name="small", bufs=24) as psm,
            tc.tile_pool(name="ppg", bufs=2, space="PSUM") as ppg,
            tc.tile_pool(name="ppc", bufs=2, space="PSUM") as ppc,
        ):
            wg_sb = pc.tile([128, NGT, 128], f8)
            nc.sync.dma_start(wg_sb[:, :, :].rearrange("p a c -> p (a c)"), wg_d[:, :])
            wc_sb = pc.tile([128, NCT, 128], f8)
            nc.sync.dma_start(wc_sb[:, :, :].rearrange("p a c -> p (a c)"), wc_d[:, :])

            sT = pc.tile([128, 8, BL], f32)
            nc.sync.dma_start(sT[:, :, :], soft_t.rearrange("(nh p) b -> p nh b", p=128))
            nc.sync.dma_start(out[0][:, :], soft_t[:, :])
            c1 = pc.tile([128, 8, BL], f32)
            nc.any.tensor_scalar(c1[:, :, :], sT[:, :, :], sp1, None, Alu.mult)
            c2 = pc.tile([128, 8, BL], f32)
            nc.any.tensor_scalar(c2[:, :, :], sT[:, :, :], sp2, None, Alu.mult)

            def split_tri(src_f32, tri, nmid):
                """tri[:, :, 0..2, :] = bf16 triple of src (h, m, l); mixed-dtype
                subtracts skip the f32 upcast copies."""
                nc.vector.tensor_copy(tri[:, :, 0, :], src_f32)
                r = pw.tile([128, nmid, BL], f32, tag=f"r{nmid}", name="r")
                nc.vector.tensor_tensor(r[:, :, :], src_f32, tri[:, :, 0, :], Alu.subtract)
                nc.vector.tensor_copy(tri[:, :, 1, :], r[:, :, :])
                nc.vector.tensor_tensor(tri[:, :, 2, :], r[:, :, :], tri[:, :, 1, :], Alu.subtract)

            def do_gather(temp_tri):
                vcA = ppg.tile([128, 12, 3, BL], f32, tag="vcA", name="vcA")
                vcB = ppg.tile([128, 12, 3, BL], f32, tag="vcB", name="vcB")
                for q in range(24):
                    dst = vcA if q < 12 else vcB
                    o = dst[:, q % 12, :, :].rearrange("p t b -> p (t b)")
                    ks = pres_q[q]
                    for i, k in enumerate(ks):
                        nc.tensor.matmul(
                            o, wg_sb[:, gt_pos[(q, k)], :],
                            temp_tri[:, k, :, :].rearrange("p t b -> p (t b)"),
                            start=(i == 0), stop=(i == len(ks) - 1))
                return vcA, vcB

            def do_colsum(cv_tri):
                cs_ps = ppc.tile([128, 8, 3, BL], f32, tag="csps", name="cs_ps")
                for k in range(8):
                    o = cs_ps[:, k, :, :].rearrange("p t b -> p (t b)")
                    qs = pres_k[k]
                    for i, q in enumerate(qs):
                        nc.tensor.matmul(
                            o, wc_sb[:, ct_pos[(k, q)], :],
                            cv_tri[:, q, :, :].rearrange("p t b -> p (t b)"),
                            start=(i == 0), stop=(i == len(qs) - 1))
                return cs_ps

            def pl(t, i, n=1):
                """n plane-groups of 4 starting at plane i."""
                return t[:, 4 * i:4 * (i + n), :]

            def g3(t, gidx):
                """planes (gidx, gidx+3) as [128, 2, 4, BL] (stride-3 pair)."""
                return t[:, :, :].rearrange("p (two g m) b -> p two g m b", two=2, g=3)[:, :, gidx, :, :]

            def w2(t, i):
                """planes (i, i+2) as [128, 2, 4, BL] (stride-2 pair window)."""
                return t[:, 4 * i:4 * i + 16, :].rearrange(
                    "p (two g m) b -> p two g m b", two=2, g=2)[:, :, 0, :, :]

            # ---- iteration 1 entry: temp = sp1 * soft ----
            temp_tri = pw.tile([128, 8, 3, BL], bf16, tag="ttri", name="ttri0")
            split_tri(c1[:, :, :], temp_tri, 8)

            cv = None
            for it in range(1, NUM_ITERS + 1):
                vcA, vcB = do_gather(temp_tri)
                vc = pw.tile([128, 24, BL], f32, tag="vc", name="vc")
                for half, ps_t in ((0, vcA), (1, vcB)):
                    sl = slice(12 * half, 12 * half + 12)
                    g1 = pw.tile([128, 12, BL], f32, tag=f"g1h{half}", name="g1")
                    if cv is None:
                        nc.vector.tensor_copy(g1[:, :, :], ps_t[:, :, 0, :])
                    else:
                        nc.vector.tensor_tensor(g1[:, :, :], ps_t[:, :, 0, :], cv[:, sl, :], Alu.subtract)
                    g2 = pw.tile([128, 12, BL], f32, tag=f"g2h{half}", name="g2")
                    nc.vector.tensor_tensor(g2[:, :, :], g1[:, :, :], ps_t[:, :, 1, :], Alu.add)
                    nc.vector.tensor_tensor(vc[:, sl, :], g2[:, :, :], ps_t[:, :, 2, :], Alu.add)

                # ---- vector phase ----
                a = pw.tile([128, 24, BL], f32, tag="a")
                nc.scalar.activation(a[:, :, :], vc[:, :, :], ActF.Abs)
                sg = pw.tile([128, 24, BL], f32, tag="sg")
                nc.scalar.activation(sg[:, :, :], vc[:, :, :], ActF.Sign)

                # psign on gpsimd (parallel with DVE sort)
                p1 = psm.tile([128, 12, BL], f32, tag="p1")
                nc.gpsimd.tensor_tensor(p1[:, :, :], sg[:, 0:12, :], sg[:, 12:24, :], Alu.mult)
                p2 = psm.tile([128, 4, BL], f32, tag="p2")
                nc.gpsimd.tensor_tensor(p2[:, :, :], p1[:, 0:4, :], p1[:, 4:8, :], Alu.mult)
                ps = psm.tile([128, 4, BL], f32, tag="ps")
                nc.gpsimd.tensor_tensor(ps[:, :, :], p2[:, :, :], p1[:, 8:12, :], Alu.mult)

                # ---- fused 5-layer sort (physical planes hold LOGMAP lanes) ----
                T1 = psrt.tile([128, 24, BL], f32, tag="T1", name="T1")
                nc.vector.tensor_tensor(pl(T1, 0, 3), pl(a, 0, 3), pl(a, 3, 3), Alu.min)
                nc.vector.tensor_tensor(pl(T1, 3, 3), pl(a, 0, 3), pl(a, 3, 3), Alu.max)
                # T1 planes = [pos0, pos1, pos2, pos5, pos3, pos4]
                T2 = psrt.tile([128, 24, BL], f32, tag="T2", name="T2")
                nc.vector.tensor_tensor(w2(T2, 1), g3(T1, 1), g3(T1, 2), Alu.min)
                nc.vector.tensor_tensor(w2(T2, 2), g3(T1, 1), g3(T1, 2), Alu.max)
                # T2 planes (1..4) = [pos1, pos2, pos3, pos4]; pos0 @ T1[0], pos5 @ T1[3]
                T3 = psrt.tile([128, 24, BL], f32, tag="T3", name="T3")
                nc.vector.tensor_tensor(pl(T3, 0), pl(T1, 0), pl(T2, 3), Alu.min)
                nc.vector.tensor_tensor(pl(T3, 4), pl(T1, 0), pl(T2, 3), Alu.max)
                nc.vector.tensor_tensor(pl(T3, 1), pl(T2, 2), pl(T1, 3), Alu.min)
                nc.vector.tensor_tensor(pl(T3, 5), pl(T2, 2), pl(T1, 3), Alu.max)
                nc.vector.tensor_copy(pl(T3, 2), pl(T2, 4))
                nc.vector.tensor_copy(pl(T3, 3), pl(T2, 1))
                # T3 planes = [pos0, pos2, pos4, pos1, pos3, pos5]
                T4 = psrt.tile([128, 24, BL], f32, tag="T4", name="T4")
                nc.vector.tensor_tensor(pl(T4, 0, 3), pl(T3, 0, 3), pl(T3, 3, 3), Alu.min)
                nc.vector.tensor_tensor(pl(T4, 3, 3), pl(T3, 0, 3), pl(T3, 3, 3), Alu.max)
                S13 = psrt.tile([128, 8, BL], f32, tag="S13", name="S13")
                nc.vector.tensor_tensor(S13[:, :, :], pl(T4, 3, 2), pl(T4, 1, 2), Alu.min)
                S24 = psrt.tile([128, 8, BL], f32, tag="S24", name="S24")
                nc.vector.tensor_tensor(S24[:, :, :], pl(T4, 3, 2), pl(T4, 1, 2), Alu.max)
                lanes = [pl(T4, 0), S13[:, 0:4, :], S24[:, 0:4, :],
                         S13[:, 4:8, :], S24[:, 4:8, :], pl(T4, 5)]

                # u_k = w_k s_k (Act, scaled copies); base tree on any
                u = []
                for kk in range(5):
                    uk = psm.tile([128, 4, BL], f32, tag=f"u{kk}", name=f"uk{kk}")
                    nc.scalar.activation(uk[:, :, :], lanes[kk], ActF.Copy, scale=w[kk])
                    u.append(uk)
                b01 = psm.tile([128, 4, BL], f32, tag="b01")
                nc.any.tensor_tensor(b01[:, :, :], u[0][:, :, :], u[1][:, :, :], Alu.add)
                b23 = psm.tile([128, 4, BL], f32, tag="b23")
                nc.any.tensor_tensor(b23[:, :, :], u[2][:, :, :], u[3][:, :, :], Alu.add)
                b03 = psm.tile([128, 4, BL], f32, tag="b03")
                nc.any.tensor_tensor(b03[:, :, :], b01[:, :, :], b23[:, :, :], Alu.add)
                base = psm.tile([128, 4, BL], f32, tag="base")
                nc.any.tensor_tensor(base[:, :, :], b03[:, :, :], u[4][:, :, :], Alu.add)

                # e_k = w_k (s_{k+1} - s_k): diff on DVE, scale on Act
                e = []
                for kk in range(5):
                    dk = psm.tile([128, 4, BL], f32, tag=f"d{kk}", name=f"dk{kk}")
                    nc.vector.tensor_tensor(dk[:, :, :], lanes[kk + 1], lanes[kk], Alu.subtract)
                    ek = psm.tile([128, 4, BL], f32, tag=f"e{kk}", name=f"ek{kk}")
                    nc.scalar.activation(ek[:, :, :], dk[:, :, :], ActF.Copy, scale=w[kk])
                    e.append(ek)

                a4 = a[:, :, :].rearrange("p (j m) b -> p j m b", j=DC)
                bshape = [128, DC, 4, BL]
                terms = []
                for kk in range(5):
                    cmp = pw.tile([128, 24, BL], f32, tag=f"cmp{kk}", name=f"cmp{kk}")
                    cmp4 = cmp[:, :, :].rearrange("p (j m) b -> p j m b", j=DC)
                    sk_b = lanes[kk].unsqueeze(1).broadcast_to(bshape)
                    nc.vector.tensor_tensor(cmp4, sk_b, a4, Alu.is_ge)
                    ek_b = e[kk][:, :, :].unsqueeze(1).broadcast_to(bshape)
                    nc.vector.tensor_tensor(cmp4, cmp4, ek_b, Alu.mult)
                    terms.append(cmp)
                t01 = pw.tile([128, 24, BL], f32, tag="t01")
                nc.vector.tensor_tensor(t01[:, :, :], terms[0][:, :, :], terms[1][:, :, :], Alu.add)
                t23 = pw.tile([128, 24, BL], f32, tag="t23")
                nc.vector.tensor_tensor(t23[:, :, :], terms[2][:, :, :], terms[3][:, :, :], Alu.add)
                t4b = pw.tile([128, 24, BL], f32, tag="t4b")
                t4b4 = t4b[:, :, :].rearrange("p (j m) b -> p j m b", j=DC)
                nc.vector.tensor_tensor(
                    t4b4, terms[4][:, :, :].rearrange("p (j m) b -> p j m b", j=DC),
                    base[:, :, :].unsqueeze(1).broadcast_to(bshape), Alu.add)
                t0123 = pw.tile([128, 24, BL], f32, tag="t0123")
                nc.vector.tensor_tensor(t0123[:, :, :], t01[:, :, :], t23[:, :, :], Alu.add)
                acc = pw.tile([128, 24, BL], f32, tag="acc")
                nc.vector.tensor_tensor(acc[:, :, :], t0123[:, :, :], t4b[:, :, :], Alu.add)

                # sg_loo = sg * psign on gpsimd (off the DVE chain)
                sg_loo = pw.tile([128, 24, BL], f32, tag="sgloo")
                sgl4 = sg_loo[:, :, :].rearrange("p (j m) b -> p j m b", j=DC)
                sg4 = sg[:, :, :].rearrange("p (j m) b -> p j m b", j=DC)
                ps_b = ps[:, :, :].unsqueeze(1).broadcast_to(bshape)
                nc.gpsimd.tensor_tensor(sgl4, sg4, ps_b, Alu.mult)
                cv = pw.tile([128, 24, BL], f32, tag="cv", name="cv")
                nc.vector.tensor_tensor(cv[:, :, :], acc[:, :, :], sg_loo[:, :, :], Alu.mult)

                # ---- split + colsum ----
                cv_tri = pw.tile([128, 24, 3, BL], bf16, tag="cvtri", name="cv_tri")
                split_tri(cv[:, :, :], cv_tri, 24)
                cs_ps = do_colsum(cv_tri)

                csh = pw.tile([128, 8, BL], f32, tag="csh", name="csh")
                nc.vector.tensor_copy(csh[:, :, :], cs_ps[:, :, 0, :])
                csm = pw.tile([128, 8, BL], f32, tag="csm", name="csm")
                nc.vector.tensor_tensor(csm[:, :, :], csh[:, :, :], cs_ps[:, :, 1, :], Alu.add)
                cs = pw.tile([128, 8, BL], f32, tag="cs", name="cs")
                nc.vector.tensor_tensor(cs[:, :, :], csm[:, :, :], cs_ps[:, :, 2, :], Alu.add)

                so = pw.tile([128, 8, BL], f32, tag="so", name="so")
                nc.any.tensor_tensor(so[:, :, :], cs[:, :, :], c2[:, :, :], Alu.add)
                nc.sync.dma_start(out[it].rearrange("(nh p) b -> p nh b", p=128), so[:, :, :])

                if it < NUM_ITERS:
                    tp = pw.tile([128, 8, BL], f32, tag="tp", name="tp")
                    nc.vector.tensor_tensor(tp[:, :, :], cs[:, :, :], c1[:, :, :], Alu.add)
                    temp_tri = pw.tile([128, 8, 3, BL], bf16, tag="ttri", name="ttri")
                    split_tri(tp[:, :, :], temp_tri, 8)

    nc.compile()
    return nc


def _get_nc(row_cols, W1, W2, bit_w1, bit_w2):
    row_cols = np.asarray(row_cols)
    w = (np.asarray(W1, np.float32) @ np.asarray(W2, np.float32))[:, 0]
    sp1 = float(np.log1p(np.exp(np.asarray(bit_w1, np.float32)))[0])
    sp2 = float(np.log1p(np.exp(np.asarray(bit_w2, np.float32)))[0])
    key = (row_cols.tobytes(), w.tobytes(), sp1, sp2)
    if key not in _CACHE:
        import ml_dtypes
        assign, cols, present, gt, ct = _layout(row_cols)
        wg, wc = _weights(cols, gt, ct)
        f8 = ml_dtypes.float8_e4m3fn
        _CACHE[key] = (_build(cols, w, sp1, sp2, gt, ct),
                       np.ascontiguousarray(wg.reshape(128, -1).astype(f8)),
                       np.ascontiguousarray(wc.reshape(128, -1).astype(f8)))
    return _CACHE[key]


def kernel(**inputs):
    from concourse.bass_utils import run_bass_kernel_spmd

    soft = np.asarray(inputs["soft_input"], np.float32)
    nc, wg, wc = _get_nc(inputs["row_cols"], inputs["W1"], inputs["W2"],
                         inputs["bit_w1"], inputs["bit_w2"])

    in_maps = []
    for c in range(NCORES):
        shard = soft[c * BL:(c + 1) * BL, :]  # [8, 1024]
        in_maps.append({
            "soft_t": np.ascontiguousarray(shard.T),  # [1024, 8]
            "wg": wg,
            "wc": wc,
        })
    res = run_bass_kernel_spmd(nc, in_maps, core_ids=list(range(NCORES)))

    full = np.empty((NUM_ITERS + 1, B, N), np.float32)
    for c in range(NCORES):
        o = res.results[c]["out"]  # [6, 1024, 8]
        full[:, c * BL:(c + 1) * BL, :] = o.transpose(0, 2, 1)
    return full

